# revision 1
# baseline (speedup 1.0000x reference)
"""Trainium2 Bass kernel for nn_LocalContrastiveLoss.

Strategy (data-parallel over B, 1 image per core, 8 cores):
  - Host re-lays-out inputs per image:
      * embeddings [E=64, HW=65536] -> transposed bf16 tiles so that pixel
        chunks of 128 land on SBUF partitions: [NG=16, 128, NCG=32 * 64]
      * labels (argmax of one-hot masks) -> pre-transposed [128, 512] bf16
      * z = sampled pixel embeddings [32, 64] f32 (pure gather, done on host)
      * sel = one-hot [32, 8] selecting each sample's own class column
  - Device per core:
      * build one-hot maskT planes [128, k, 512] from labels via is_equal
      * 512 accumulating matmuls: lhsT=maskT chunk [128,8], rhs=embT chunk
        [128,64] -> PSUM [8,64] = per-class embedding sums over all pixels
        (division by counts cancels under cosine normalization)
      * normalize class means and z rows (fold 1/TEMP into z), transpose the
        two small matrices via PE, sims = znT.T @ mnT -> [32, 8]
      * logsumexp over classes, s_pos via sel mask, per-core partial sum
  - Host: mean of the 8 partial sums / 256.
"""

import numpy as np
import ml_dtypes

import concourse.bass as bass
import concourse.bacc as bacc
import concourse.tile as tile
from concourse import mybir
from concourse.bass_utils import run_bass_kernel_spmd
from concourse.masks import make_identity

B, E, H, W, K, NPOS = 8, 64, 256, 256, 8, 4
HW = H * W
TEMP = 0.2
EPS = 1e-8
NCHUNK = HW // 128          # 512 chunks of 128 pixels
NCG = 32                    # chunks per DMA group
NG = NCHUNK // NCG          # 16 groups
NJ = K * NPOS               # 32 sampled pixels per image

f32 = mybir.dt.float32
bf16 = mybir.dt.bfloat16


def build_bass():
    nc = bacc.Bacc(None, target_bir_lowering=False)

    embT = nc.dram_tensor("embT", [NG, 128, NCG * E], bf16, kind="ExternalInput")
    labT = nc.dram_tensor("labT", [128, NCHUNK], bf16, kind="ExternalInput")
    z_in = nc.dram_tensor("z", [NJ, E], f32, kind="ExternalInput")
    sel_in = nc.dram_tensor("sel", [NJ, K], f32, kind="ExternalInput")
    out = nc.dram_tensor("out", [1, 1], f32, kind="ExternalOutput")

    with tile.TileContext(nc) as tc:
        with (
            tc.tile_pool(name="big", bufs=NG) as big,
            tc.tile_pool(name="planes", bufs=1) as planesp,
            tc.tile_pool(name="small", bufs=1) as small,
            tc.tile_pool(name="psum", bufs=1, space="PSUM") as psum,
        ):
            # --- labels -> one-hot maskT planes [128, K, NCHUNK] bf16
            lab_t = small.tile([128, NCHUNK], bf16)
            nc.sync.dma_start(out=lab_t, in_=labT[:, :])
            planes = planesp.tile([128, K, NCHUNK], bf16)
            for k in range(K):
                nc.vector.tensor_scalar(
                    out=planes[:, k, :],
                    in0=lab_t[:, :],
                    scalar1=float(k),
                    scalar2=None,
                    op0=mybir.AluOpType.is_equal,
                )

            # --- identity for PE transposes, ones for partition reduction
            ident = small.tile([NJ, NJ], f32)
            make_identity(nc, ident)
            ones = small.tile([NJ, 1], f32)
            nc.vector.memset(ones, 1.0)

            # --- 512 accumulating matmuls: class sums [K, E].
            # Pack 4 consecutive chunks into the 4 PE column-groups
            # (tile_position) so they execute concurrently; each group
            # accumulates into its own 32-partition PSUM slice.
            means_ps = psum.tile([128, E], f32)
            for g in range(NG):
                et = big.tile([128, NCG * E], bf16)
                nc.sync.dma_start(out=et, in_=embT[g, :, :])
                for cl in range(NCG):
                    c = g * NCG + cl
                    j = c % 4
                    nc.tensor.matmul(
                        means_ps[32 * j:32 * j + K, :],
                        planes[:, :, c],
                        et[:, cl * E:(cl + 1) * E],
                        start=(c < 4),
                        stop=(c >= NCHUNK - 4),
                        tile_position=(0, 32 * j),
                    )

            # --- normalize class means (count division cancels in cosine)
            # sum the 4 column-group accumulators (only one PSUM src per op)
            m_sb = small.tile([K, E], f32)
            nc.vector.tensor_copy(m_sb, means_ps[0:K, :])
            nc.vector.tensor_add(m_sb, m_sb, means_ps[32:32 + K, :])
            nc.vector.tensor_add(m_sb, m_sb, means_ps[64:64 + K, :])
            nc.vector.tensor_add(m_sb, m_sb, means_ps[96:96 + K, :])
            msq = small.tile([K, E], f32)
            nc.vector.tensor_mul(msq, m_sb, m_sb)
            mnrm = small.tile([K, 1], f32)
            nc.vector.tensor_reduce(
                mnrm, msq, axis=mybir.AxisListType.X, op=mybir.AluOpType.add
            )
            nc.scalar.activation(mnrm, mnrm, mybir.ActivationFunctionType.Sqrt)
            nc.vector.tensor_scalar_max(mnrm, mnrm, EPS)
            mrinv = small.tile([K, 1], f32)
            nc.vector.reciprocal(mrinv, mnrm)
            mn = small.tile([K, E], f32)
            nc.vector.tensor_scalar_mul(mn, m_sb, mrinv)

            # --- normalize z rows, fold in 1/TEMP
            z_sb = small.tile([NJ, E], f32)
            nc.sync.dma_start(out=z_sb, in_=z_in[:, :])
            zsq = small.tile([NJ, E], f32)
            nc.vector.tensor_mul(zsq, z_sb, z_sb)
            znrm = small.tile([NJ, 1], f32)
            nc.vector.tensor_reduce(
                znrm, zsq, axis=mybir.AxisListType.X, op=mybir.AluOpType.add
            )
            nc.scalar.activation(znrm, znrm, mybir.ActivationFunctionType.Sqrt)
            nc.vector.tensor_scalar_max(znrm, znrm, EPS)
            zrinv = small.tile([NJ, 1], f32)
            nc.vector.reciprocal(zrinv, znrm)
            zn = small.tile([NJ, E], f32)
            nc.vector.tensor_scalar(
                out=zn,
                in0=z_sb,
                scalar1=zrinv,
                scalar2=1.0 / TEMP,
                op0=mybir.AluOpType.mult,
                op1=mybir.AluOpType.mult,
            )

            # --- transpose both small matrices via PE (need E on partitions)
            mnT_ps = psum.tile([E, K], f32)
            nc.tensor.transpose(mnT_ps, mn, ident[:K, :K])
            mnT = small.tile([E, K], f32)
            nc.vector.tensor_copy(mnT, mnT_ps)
            znT_ps = psum.tile([E, NJ], f32)
            nc.tensor.transpose(znT_ps, zn, ident[:, :])
            znT = small.tile([E, NJ], f32)
            nc.vector.tensor_copy(znT, znT_ps)

            # --- sims[j, k] = zn[j] . mn[k]  (already scaled by 1/TEMP)
            sims_ps = psum.tile([NJ, K], f32)
            nc.tensor.matmul(sims_ps, znT, mnT, start=True, stop=True)
            sims = small.tile([NJ, K], f32)
            nc.vector.tensor_copy(sims, sims_ps)

            # --- logsumexp over classes + positive term
            mx = small.tile([NJ, 1], f32)
            nc.vector.tensor_reduce(
                mx, sims, axis=mybir.AxisListType.X, op=mybir.AluOpType.max
            )
            nmx = small.tile([NJ, 1], f32)
            nc.vector.tensor_scalar_mul(nmx, mx, -1.0)
            ex = small.tile([NJ, K], f32)
            nc.scalar.activation(
                ex, sims, mybir.ActivationFunctionType.Exp, bias=nmx, scale=1.0
            )
            sm = small.tile([NJ, 1], f32)
            nc.vector.tensor_reduce(
                sm, ex, axis=mybir.AxisListType.X, op=mybir.AluOpType.add
            )
            den = small.tile([NJ, 1], f32)
            nc.scalar.activation(den, sm, mybir.ActivationFunctionType.Ln)

            sel_sb = small.tile([NJ, K], f32)
            nc.sync.dma_start(out=sel_sb, in_=sel_in[:, :])
            spt = small.tile([NJ, K], f32)
            nc.vector.tensor_mul(spt, sims, sel_sb)
            sp = small.tile([NJ, 1], f32)
            nc.vector.tensor_reduce(
                sp, spt, axis=mybir.AxisListType.X, op=mybir.AluOpType.add
            )

            # loss_j = den + mx - sp
            loss = small.tile([NJ, 1], f32)
            nc.vector.tensor_add(loss, den, mx)
            nc.vector.tensor_tensor(
                out=loss, in0=loss, in1=sp, op=mybir.AluOpType.subtract
            )

            # --- partial sum over the 32 rows via ones-matmul
            tot_ps = psum.tile([1, 1], f32)
            nc.tensor.matmul(tot_ps, loss, ones, start=True, stop=True)
            tot = small.tile([1, 1], f32)
            nc.vector.tensor_copy(tot, tot_ps)
            nc.sync.dma_start(out=out[:, :], in_=tot)

    if not nc.is_finalized():
        nc.finalize()
    return nc


def _prep_inputs(embeddings, masks_onehot, pos_pix):
    embf = np.ascontiguousarray(
        np.asarray(embeddings, dtype=np.float32).reshape(B, E, HW)
    )
    m = np.asarray(masks_onehot, dtype=np.float32).reshape(B, K, HW)
    labels = np.argmax(m, axis=1)  # [B, HW], exact one-hot

    # embT grouped: [B, NG, 128, NCG*E] bf16, partition = pixel-within-chunk
    embT = embf.transpose(0, 2, 1).reshape(B, NG, NCG, 128, E)
    embT = np.ascontiguousarray(embT.transpose(0, 1, 3, 2, 4)).reshape(
        B, NG, 128, NCG * E
    ).astype(ml_dtypes.bfloat16)

    # labT: [B, 128, NCHUNK] bf16 (labels reshaped [NCHUNK,128] then transposed)
    labT = np.ascontiguousarray(
        labels.reshape(B, NCHUNK, 128).transpose(0, 2, 1)
    ).astype(ml_dtypes.bfloat16)

    # z gather (host): [B, NJ, E] f32
    pix = np.asarray(pos_pix).reshape(B, NJ)
    z = np.stack([embf[b][:, pix[b]].T for b in range(B)]).astype(np.float32)

    sel = np.zeros((NJ, K), dtype=np.float32)
    sel[np.arange(NJ), np.arange(NJ) // NPOS] = 1.0

    return [
        {
            "embT": np.ascontiguousarray(embT[b]),
            "labT": np.ascontiguousarray(labT[b]),
            "z": np.ascontiguousarray(z[b]),
            "sel": sel,
        }
        for b in range(B)
    ]


def _run(embeddings, masks_onehot, pos_pix, trace=False):
    in_maps = _prep_inputs(embeddings, masks_onehot, pos_pix)
    nc = build_bass()
    res = run_bass_kernel_spmd(nc, in_maps, core_ids=list(range(B)), trace=trace)
    partials = [np.asarray(r["out"], dtype=np.float64)[0, 0] for r in res.results]
    total = sum(partials) / float(B * K * NPOS)
    return np.float32(total), res


def kernel(embeddings, masks_onehot, pos_pix):
    val, _ = _run(embeddings, masks_onehot, pos_pix)
    return np.asarray(val, dtype=np.float32)

